# revision 1
# baseline (speedup 1.0000x reference)
"""v3: phase-shrunk schedule. See v2 docstring (kernel_v2.py) for the core
algorithm. Changes vs v2:
 - DMAs spread across three queues (SP-HWDGE, ACT-HWDGE, Pool-SWDGE) —
   v2 serialized all transfers on one queue (~23us for each 8MB load)
 - prologue holds only K/Q-chunk0/V-chunk0 projections + 8 V-transposes;
   V-chunk1 + remaining transposes weave into attention qc0 (borrowing the
   ctx PSUM tags while ctx accumulation is deferred into a deeper exp pool);
   Q-chunk1 is emitted between the qc super-iterations
 - batch-1 load is emitted from inside batch-0's attention (after the last
   qt reader), overlapping the transfer with compute
 - epilogue outproj evacuations alternate DVE/ACT
"""

import functools
from collections import deque
from contextlib import ExitStack

import numpy as np

import concourse.bass as bass
import concourse.tile as tile
from concourse import mybir
from concourse.bass_utils import run_bass_kernel_spmd

B, S, D, H, DH = 2, 2048, 1024, 16, 64
N_CORES = 8
DPC = D // N_CORES
BS = B * S
NQC = S // 1024           # 2
NST = S // 128            # 16
NKT = D // 128            # 8

F32 = mybir.dt.float32
F32R = mybir.dt.float32r
Act = mybir.ActivationFunctionType
Alu = mybir.AluOpType


def _split_sync_commands(nc, max_waits=1, max_updates=8):
    for fn in nc.m.functions:
        for bb in fn.blocks:
            new_insts = []
            changed = False
            for inst in bb.instructions:
                si = getattr(inst, "sync_info", None)
                if si is not None:
                    waits = list(si.on_wait or [])
                    if len(waits) > max_waits:
                        for w in waits[:-max_waits]:
                            new_insts.append(mybir.InstNoOp(
                                name=nc.get_next_instruction_name(),
                                ins=[], outs=[], engine=inst.engine,
                                sync_info=mybir.SyncInfo(on_wait=[w], on_update=[]),
                            ))
                        si.on_wait = waits[-max_waits:]
                        changed = True
                    updates = list(si.on_update or [])
                    if len(updates) > max_updates:
                        si.on_update = updates[:max_updates]
                        new_insts.append(inst)
                        new_insts.append(mybir.InstNoOp(
                            name=nc.get_next_instruction_name(),
                            ins=[], outs=[], engine=inst.engine,
                            sync_info=mybir.SyncInfo(
                                on_wait=[], on_update=updates[max_updates:]),
                        ))
                        changed = True
                        continue
                new_insts.append(inst)
            if changed:
                bb.instructions = new_insts


def _bcast_rows(ap, nrows):
    return bass.AP(tensor=ap.tensor, offset=ap.offset,
                   ap=[[0, nrows]] + [list(p) for p in ap.ap[1:]])


@functools.lru_cache(maxsize=1)
def _build():
    nc = bass.Bass()
    qt_d = nc.dram_tensor("qt", [D, BS], F32, kind="ExternalInput")
    wq_d = nc.dram_tensor("wq", [D, DPC], F32, kind="ExternalInput")
    wk_d = nc.dram_tensor("wk", [D, DPC], F32, kind="ExternalInput")
    wv_d = nc.dram_tensor("wv", [D, DPC], F32, kind="ExternalInput")
    bq_d = nc.dram_tensor("bq", [DPC, 1], F32, kind="ExternalInput")
    bk_d = nc.dram_tensor("bk", [DPC, 1], F32, kind="ExternalInput")
    bv_d = nc.dram_tensor("bv", [DPC, 1], F32, kind="ExternalInput")
    wo_d = nc.dram_tensor("wo", [DPC, D], F32, kind="ExternalInput")
    out_d = nc.dram_tensor("out_part", [BS, D], F32, kind="ExternalOutput")
    dn_d = nc.dram_tensor("dn_scratch", [2, S], F32)
    ident_d = nc.inline_tensor(np.eye(128, dtype=np.float32), "ident")
    ones_d = nc.inline_tensor(np.ones((1, 1), dtype=np.float32), "ones_const")

    with tile.TileContext(nc) as tc, ExitStack() as ctx:
        consts = ctx.enter_context(tc.tile_pool(name="consts", bufs=1))
        qt_pool = ctx.enter_context(tc.tile_pool(name="qt", bufs=1))
        proj = ctx.enter_context(tc.tile_pool(name="proj", bufs=2))
        vpool = ctx.enter_context(tc.tile_pool(name="vpool", bufs=2))
        vtp = ctx.enter_context(tc.tile_pool(name="vtp", bufs=1))
        ctxp = ctx.enter_context(tc.tile_pool(name="ctxp", bufs=2))
        expp = ctx.enter_context(tc.tile_pool(name="expp", bufs=4))
        dnp = ctx.enter_context(tc.tile_pool(name="dnp", bufs=1))
        outp = ctx.enter_context(tc.tile_pool(name="outp", bufs=3))
        psp = ctx.enter_context(tc.tile_pool(name="psp", bufs=1, space="PSUM"))

        def ps_tile(shape, tag):
            return psp.tile(shape, F32, tag=tag, name="ps_" + tag)

        # ---- constants (weights via the Pool SWDGE queue: off the qt path) --
        wq_sb = consts.tile([128, NKT, DPC], F32R, tag="wq")
        wk_sb = consts.tile([128, NKT, DPC], F32R, tag="wk")
        wv_sb = consts.tile([128, NKT, DPC], F32R, tag="wv")
        for k in range(NKT):
            nc.sync.dma_start(out=wk_sb[:, k, :], in_=wk_d[k * 128:(k + 1) * 128, :].bitcast(F32R))
            nc.scalar.dma_start(out=wq_sb[:, k, :], in_=wq_d[k * 128:(k + 1) * 128, :].bitcast(F32R))
            nc.sync.dma_start(out=wv_sb[:, k, :], in_=wv_d[k * 128:(k + 1) * 128, :].bitcast(F32R))
        wo_sb = consts.tile([128, D], F32R, tag="wo")
        nc.gpsimd.dma_start(out=wo_sb, in_=wo_d[:, :].bitcast(F32R))
        bq_sb = consts.tile([128, 1], F32, tag="bq")
        bk_sb = consts.tile([128, 1], F32, tag="bk")
        bv_sb = consts.tile([128, 1], F32, tag="bv")
        nc.gpsimd.dma_start(out=bq_sb, in_=bq_d[:, :])
        nc.gpsimd.dma_start(out=bk_sb, in_=bk_d[:, :])
        nc.gpsimd.dma_start(out=bv_sb, in_=bv_d[:, :])
        ident_sb = consts.tile([128, 128], F32, tag="ident")
        nc.gpsimd.dma_start(out=ident_sb, in_=ident_d[:, :])
        # (wo + biases + ident ride the idle Pool queue: not on the critical path)
        eighth_sb = consts.tile([128, 1], F32, tag="eighth")
        nc.vector.memset(eighth_sb, 0.125)
        one_sb = consts.tile([128, 1], F32, tag="one")
        nc.vector.memset(one_sb, 1.0)
        zero_sb = consts.tile([128, 1], F32, tag="zero")
        nc.vector.memset(zero_sb, 0.0)

        state = {}

        def load(b, engines):
            """qt load spread over 2 DMA queues, k-major so early k tiles
            land first. Never put scalar-queue (ACT-issued) DMAs where the
            ACT engine is busy — a full queue blocks the ACT sequencer."""
            qt_sb = qt_pool.tile([128, NKT, S], F32R, tag="qt")
            i = 0
            for h in range(4):      # h-major: chunk-0 projections unblock first
                for k in range(NKT):
                    engines[i % len(engines)].dma_start(
                        out=qt_sb[:, k, h * 512:(h + 1) * 512],
                        in_=qt_d[k * 128:(k + 1) * 128,
                                 b * S + h * 512: b * S + (h + 1) * 512].bitcast(F32R))
                    i += 1
            state[b, "qt"] = qt_sb

        def proj_chunk(b, which, pc, tag):
            qt_sb = state[b, "qt"]
            w_sb, b_sb, sc_sb = {
                "q": (wq_sb, bq_sb, eighth_sb),
                "k": (wk_sb, bk_sb, one_sb),
                "v": (wv_sb, bv_sb, one_sb),
            }[which]
            dst = state[b, {"q": "QT", "k": "KT", "v": "VT"}[which]]
            ps = ps_tile([128, 1024], tag)
            for k in range(NKT):
                for hh in range(2):
                    nc.tensor.matmul(
                        ps[:, hh * 512:(hh + 1) * 512], w_sb[:, k, :],
                        qt_sb[:, k, pc * 1024 + hh * 512: pc * 1024 + (hh + 1) * 512],
                        start=(k == 0), stop=(k == NKT - 1))
            nc.vector.tensor_scalar(
                out=dst[:, pc * 1024:(pc + 1) * 1024], in0=ps,
                scalar1=b_sb, scalar2=sc_sb, op0=Alu.add, op1=Alu.mult)

        def alloc_proj(b):
            state[b, "QT"] = proj.tile([128, S], F32R, tag="QT", name="QT")
            state[b, "KT"] = proj.tile([128, S], F32R, tag="KT", name="KT")
            state[b, "VT"] = vtp.tile([128, S], F32, tag="VT", name="VT")

        def alloc_v(b):
            V = vpool.tile([128, NST, 2, DH + 1], F32R, tag="V", name="V")
            ones_ap = ones_d[:, :]
            nc.sync.dma_start(
                out=V[:, :, :, DH:DH + 1],
                in_=bass.AP(tensor=ones_ap.tensor, offset=ones_ap.offset,
                            ap=[[0, 128], [0, NST * 2], [1, 1]]).bitcast(F32R))
            state[b, "V"] = V

        def tr_one(b, st, tag):
            VT, V = state[b, "VT"], state[b, "V"]
            ps_t = ps_tile([128, 128], tag)
            nc.tensor.transpose(ps_t, VT[:, st * 128:(st + 1) * 128], ident_sb)
            for u in range(2):
                nc.vector.tensor_copy(V[:, st, u, 0:DH], ps_t[:, u * DH:(u + 1) * DH])

        def outproj_st(b, st, tag, evac_act=False, store_eng=None):
            ctxT = state[b, "ctxT"]
            o_sb = outp.tile([128, D], F32, tag="o", name="o_sb")
            ps = ps_tile([128, 1024], tag)
            for oc in range(2):
                nc.tensor.matmul(ps[:, oc * 512:(oc + 1) * 512],
                                 ctxT[:, st * 128:(st + 1) * 128],
                                 wo_sb[:, oc * 512:(oc + 1) * 512],
                                 start=True, stop=True)
            if evac_act:
                nc.scalar.activation(o_sb, ps, Act.Copy, bias=0.0, scale=1.0)
            else:
                nc.vector.tensor_copy(o_sb, ps)
            eng = store_eng or (nc.sync if st % 2 == 0 else nc.gpsimd)
            eng.dma_start(
                out=out_d[b * S + st * 128: b * S + (st + 1) * 128, :], in_=o_sb)

        def alloc_attn(b):
            state[b, "ctxT"] = ctxp.tile([128, S], F32R, tag="ctxT", name="ctxT")
            state[b, "denom"] = dnp.tile([1, 2, S], F32, tag="denom", name="denom")

        def attention_qc(b, qc, inserts=()):
            QT, KT, V = state[b, "QT"], state[b, "KT"], state[b, "V"]
            ctxT, denom = state[b, "ctxT"], state[b, "denom"]
            sl = slice(qc * 1024, (qc + 1) * 1024)
            inserts = deque(inserts)
            pcs = [None, None]
            pss = [None, None]
            pending = deque()

            def scores(u, sk):
                pss[u] = ps_tile([128, 1024], "sA" if u == 0 else "sB")
                for hh in range(2):
                    nc.tensor.matmul(
                        pss[u][:, hh * 512:(hh + 1) * 512],
                        KT[u * DH:(u + 1) * DH, sk * 128:(sk + 1) * 128],
                        QT[u * DH:(u + 1) * DH,
                           qc * 1024 + hh * 512:qc * 1024 + (hh + 1) * 512],
                        start=True, stop=True)

            def expop(u, sk):
                e = expp.tile([128, 1024], F32R, tag="exp", name="exp_t")
                nc.scalar.activation(e, pss[u], Act.Exp, bias=zero_sb, scale=1.0)
                pending.append((u, sk, e))

            def ctx_drain(target_len):
                while len(pending) > target_len:
                    u, sk, e = pending.popleft()
                    if pcs[u] is None:
                        pcs[u] = ps_tile([DH + 1, 1024], "cA" if u == 0 else "cB")
                    for hh in range(2):
                        nc.tensor.matmul(
                            pcs[u][:, hh * 512:(hh + 1) * 512], V[:, sk, u, :],
                            e[:, hh * 512:(hh + 1) * 512],
                            start=(sk == 0), stop=(sk == NST - 1))

            scores(0, 0)
            scores(1, 0)
            for sk in range(NST):
                expop(0, sk)
                expop(1, sk)
                if sk + 1 < NST:
                    scores(0, sk + 1)
                if inserts:
                    inserts.popleft()()
                if sk + 1 < NST:
                    scores(1, sk + 1)
                if inserts:
                    ctx_drain(12)
                else:
                    ctx_drain(2)
            while inserts:
                inserts.popleft()()
            ctx_drain(0)

            for u in range(2):
                nc.vector.tensor_copy(ctxT[u * DH:(u + 1) * DH, sl], pcs[u][0:DH, :])
                nc.vector.tensor_copy(denom[0:1, u, sl], pcs[u][DH:DH + 1, :])

        def normalize(b, qc=None):
            ctxT, denom = state[b, "ctxT"], state[b, "denom"]
            sl = slice(0, S) if qc is None else slice(qc * 1024, (qc + 1) * 1024)
            nc.sync.dma_start(out=dn_d[:, sl], in_=denom[0:1, :, sl])
            key = (b, "rep")
            if key not in state:
                state[key] = dnp.tile([128, S], F32, tag="rep", name="rep")
            rep = state[key]
            for u in range(2):
                nc.sync.dma_start(out=rep[u * DH:(u + 1) * DH, sl],
                                  in_=_bcast_rows(dn_d[u:u + 1, sl], DH))
            nc.vector.reciprocal(rep[:, sl], rep[:, sl])
            nc.vector.tensor_mul(ctxT[:, sl], ctxT[:, sl], rep[:, sl].bitcast(F32R))

        def thunk(f, *a):
            def g():
                f(*a)
            return g

        # =========================== schedule ===========================
        load(0, (nc.sync, nc.scalar))
        alloc_proj(0)
        alloc_v(0)
        proj_chunk(0, "k", 0, "sA")
        proj_chunk(0, "q", 0, "sB")
        proj_chunk(0, "v", 0, "sA")
        for st in range(8):
            tr_one(0, st, "cA" if st % 2 == 0 else "cB")
        proj_chunk(0, "k", 1, "sB")
        proj_chunk(0, "q", 1, "sA")
        proj_chunk(0, "v", 1, "sB")
        for st in range(8, NST):
            tr_one(0, st, "cA" if st % 2 == 0 else "cB")
        load(1, (nc.sync, nc.gpsimd))  # overlaps attn0; ACT queue untouched

        alloc_attn(0)
        attention_qc(0, 0)
        normalize(0, 0)       # overlaps attn0-qc1
        attention_qc(0, 1)
        normalize(0, 1)

        alloc_proj(1)
        alloc_v(1)
        proj_chunk(1, "k", 0, "sA")
        proj_chunk(1, "q", 0, "sB")
        proj_chunk(1, "v", 0, "sA")
        for st in range(8):
            tr_one(1, st, "cA" if st % 2 == 0 else "cB")
        proj_chunk(1, "k", 1, "sB")
        proj_chunk(1, "q", 1, "sA")
        proj_chunk(1, "v", 1, "sB")
        for st in range(8, NST):
            tr_one(1, st, "cA" if st % 2 == 0 else "cB")
        for st in range(NST):
            outproj_st(0, st, ("sA", "sB", "cA", "cB")[st % 4], evac_act=(st % 2 == 1))

        alloc_attn(1)
        attention_qc(1, 0)
        normalize(1, 0)       # overlaps attn1-qc1 (DVE/DMA only, no PE)
        attention_qc(1, 1)
        normalize(1, 1)
        for st in range(NST):
            outproj_st(1, st, ("sA", "sB", "cA", "cB")[st % 4], evac_act=(st % 2 == 1),
                       store_eng=(nc.sync if st % 2 == 0 else nc.scalar))

    _split_sync_commands(nc)
    return nc


def _prepare(query, q_w, q_b, k_w, k_b, v_w, v_b, out_w):
    qt = np.ascontiguousarray(query.reshape(BS, D).T)  # [D, BS]
    in_maps = []
    for c in range(N_CORES):
        sl = slice(c * DPC, (c + 1) * DPC)
        in_maps.append({
            "qt": qt,
            "wq": np.ascontiguousarray(q_w[sl, :].T),
            "wk": np.ascontiguousarray(k_w[sl, :].T),
            "wv": np.ascontiguousarray(v_w[sl, :].T),
            "bq": np.ascontiguousarray(q_b[sl].reshape(DPC, 1)),
            "bk": np.ascontiguousarray(k_b[sl].reshape(DPC, 1)),
            "bv": np.ascontiguousarray(v_b[sl].reshape(DPC, 1)),
            "wo": np.ascontiguousarray(out_w[:, sl].T),
        })
    return in_maps


def kernel(query, mask, q_w, q_b, k_w, k_b, v_w, v_b, out_w, out_b):
    query = np.asarray(query, dtype=np.float32)
    q_w = np.asarray(q_w, dtype=np.float32); q_b = np.asarray(q_b, dtype=np.float32)
    k_w = np.asarray(k_w, dtype=np.float32); k_b = np.asarray(k_b, dtype=np.float32)
    v_w = np.asarray(v_w, dtype=np.float32); v_b = np.asarray(v_b, dtype=np.float32)
    out_w = np.asarray(out_w, dtype=np.float32); out_b = np.asarray(out_b, dtype=np.float32)

    in_maps = _prepare(query, q_w, q_b, k_w, k_b, v_w, v_b, out_w)
    nc = _build()
    res = run_bass_kernel_spmd(nc, in_maps, core_ids=list(range(N_CORES)))
    out = np.zeros((BS, D), dtype=np.float32)
    for c in range(N_CORES):
        out += res.results[c]["out_part"]
    out += out_b[None, :]
    return out.reshape(B, S, D)



# revision 4
# speedup vs baseline: 1.1665x; 1.1665x over previous
"""v4: ACT-saturated schedule around the exp stream.

Cost-model-driven redesign vs v3 (see kernel_v3_baseline.py):
 - matmul cost = N(out free) x cycles_per_row(moving dtype); bf16 moving is
   1 cyc/row at any N (f32r needs N>=256). All HBM-sourced operands are
   pre-converted to bf16 on the host (halves load DMA too).
 - ctx matmul swapped: stationary = exp tile [keys,128q] (full 128x128),
   moving = V [keys, 64+ones] -> ctx cost halves; softmax denominator rides
   along as a ones column; normalization becomes a per-partition
   tensor_scalar at evac time.
 - V is projected directly transposed (stationary = qt tile, moving = wv):
   no PE transposes anywhere.
 - ctx^T for the out-projection via DMA-transpose (16x128 XBAR tiles).
 - k-bias dropped (exactly cancels in softmax), v-bias and out-bias folded
   on the host (attention rows sum to 1), q-bias folded into the QT evac.
 - ACT engine does nothing but the 128 exps (the roofline: ~133us); PE work
   of adjacent phases (proj, u1-ctx pass, outproj) is woven between score
   matmuls as cost-bounded inserts so the exp stream never starves. PE
   warmup matmuls defeat the p-state ramp.
 - PSUM (8 banks exactly): sA,sB [128,1024] (2+2), cA,cB [128,260] (1+1,
   u0 ctx: four 65-col qt groups each), w1,w2 [128,512] (1+1, rotating:
   warmup, k/q-proj chunks, v-proj tiles, u1-ctx qt groups, outproj halves).
"""

import functools
from collections import deque
from contextlib import ExitStack

import numpy as np
import ml_dtypes

import concourse.bass as bass
import concourse.tile as tile
from concourse import mybir
from concourse.bass_utils import run_bass_kernel_spmd

B, S, D, H, DH = 2, 2048, 1024, 16, 64
N_CORES = 8
DPC = D // N_CORES          # 128 channels/core = 2 heads
BS = B * S
NST = 16                    # key tiles of 128
NKT = 8                     # contraction tiles of 128

F32 = mybir.dt.float32
F32R = mybir.dt.float32r
BF16 = mybir.dt.bfloat16
Act = mybir.ActivationFunctionType
Alu = mybir.AluOpType
BF = ml_dtypes.bfloat16


def _split_sync_commands(nc, max_waits=1, max_updates=8):
    for fn in nc.m.functions:
        for bb in fn.blocks:
            new_insts = []
            changed = False
            for inst in bb.instructions:
                si = getattr(inst, "sync_info", None)
                if si is not None:
                    waits = list(si.on_wait or [])
                    if len(waits) > max_waits:
                        for w in waits[:-max_waits]:
                            new_insts.append(mybir.InstNoOp(
                                name=nc.get_next_instruction_name(),
                                ins=[], outs=[], engine=inst.engine,
                                sync_info=mybir.SyncInfo(on_wait=[w], on_update=[]),
                            ))
                        si.on_wait = waits[-max_waits:]
                        changed = True
                    updates = list(si.on_update or [])
                    if len(updates) > max_updates:
                        si.on_update = updates[:max_updates]
                        new_insts.append(inst)
                        new_insts.append(mybir.InstNoOp(
                            name=nc.get_next_instruction_name(),
                            ins=[], outs=[], engine=inst.engine,
                            sync_info=mybir.SyncInfo(
                                on_wait=[], on_update=updates[max_updates:]),
                        ))
                        changed = True
                        continue
                new_insts.append(inst)
            if changed:
                bb.instructions = new_insts


@functools.lru_cache(maxsize=1)
def _build():
    nc = bass.Bass()
    qt_d = nc.dram_tensor("qt", [D, BS], BF16, kind="ExternalInput")
    wq_d = nc.dram_tensor("wq", [128, NKT * DPC], BF16, kind="ExternalInput")
    wk_d = nc.dram_tensor("wk", [128, NKT * DPC], BF16, kind="ExternalInput")
    wv_d = nc.dram_tensor("wv", [128, NKT * DPC], BF16, kind="ExternalInput")
    bq_d = nc.dram_tensor("bq", [DPC, 1], F32, kind="ExternalInput")
    wo_d = nc.dram_tensor("wo", [DPC, D], BF16, kind="ExternalInput")
    out_d = nc.dram_tensor("out_part", [BS, D], BF16, kind="ExternalOutput")

    with tile.TileContext(nc) as tc, ExitStack() as ctx:
        consts = ctx.enter_context(tc.tile_pool(name="consts", bufs=1))
        qtp = ctx.enter_context(tc.tile_pool(name="qtp", bufs=1))
        proj = ctx.enter_context(tc.tile_pool(name="proj", bufs=2))
        vp = ctx.enter_context(tc.tile_pool(name="vp", bufs=2))
        expp = ctx.enter_context(tc.tile_pool(name="expp", bufs=1))
        csbp = ctx.enter_context(tc.tile_pool(name="csbp", bufs=8))
        ctp = ctx.enter_context(tc.tile_pool(name="ctp", bufs=2))
        rcpp = ctx.enter_context(tc.tile_pool(name="rcpp", bufs=4))
        outp = ctx.enter_context(tc.tile_pool(name="outp", bufs=4))
        psp = ctx.enter_context(tc.tile_pool(name="psp", bufs=1, space="PSUM"))

        def ps_tile(shape, tag):
            return psp.tile(shape, F32, tag=tag, name="ps_" + tag)

        _wrot = [0]

        def next_w():
            _wrot[0] ^= 1
            return "w1" if _wrot[0] else "w2"

        # ---------------- constants / warmup ----------------
        wconst = consts.tile([128, 640], BF16, tag="wconst")
        nc.vector.memset(wconst, 0.0)
        zero_sb = consts.tile([128, 1], F32, tag="zero")
        nc.vector.memset(zero_sb, 0.0)
        eighth_sb = consts.tile([128, 1], F32, tag="eighth")
        nc.vector.memset(eighth_sb, 0.125)

        for _ in range(6):
            ps = ps_tile([128, 512], next_w())
            nc.tensor.matmul(ps, wconst[:, 0:128], wconst[:, 128:640],
                             start=True, stop=True)

        # ---------------- weight / input loads ----------------
        # SP queue order = priority: wk, wq first, then qt b0 chunks, wv,
        # then qt b1 (DMA device drains in this order).
        wk_sb = consts.tile([128, NKT, DPC], BF16, tag="wk")
        wq_sb = consts.tile([128, NKT, DPC], BF16, tag="wq")
        wv_sb = consts.tile([128, NKT, DPC], BF16, tag="wv")
        bq_sb = consts.tile([128, 1], F32, tag="bq")
        wo_sb = consts.tile([128, D], BF16, tag="wo")
        nc.sync.dma_start(out=wk_sb, in_=wk_d[:, :])
        nc.sync.dma_start(out=wq_sb, in_=wq_d[:, :])
        nc.scalar.dma_start(out=bq_sb, in_=bq_d[:, :])
        nc.scalar.dma_start(out=wo_sb, in_=wo_d[:, :])

        state = {}

        def load_qt(b, ncol=512):
            qt_sb = qtp.tile([128, NKT, S], BF16, tag=f"qt{b}", name=f"qt{b}")
            qa = qt_d[:, :]
            for c0 in range(0, S, ncol):
                nc.sync.dma_start(
                    out=qt_sb[:, :, c0:c0 + ncol],
                    in_=bass.AP(tensor=qa.tensor,
                                offset=qa.offset + b * S + c0,
                                ap=[[BS, 128], [128 * BS, NKT], [1, ncol]]))
            state[b, "qt"] = qt_sb

        # V layout: [keys, st, 2*65]; cols u*65..u*65+63 = V_u, col u*65+64 = 1
        def alloc_v(b):
            V = vp.tile([128, NST, 130], BF16, tag="V", name="V")
            ones_ap = bass.AP(tensor=V.tensor, offset=V.offset + 64,
                              ap=[list(V.ap[0]), [130, NST], [65, 2], [1, 1]])
            nc.gpsimd.memset(ones_ap, 1.0)
            state[b, "V"] = V

        def alloc_proj(b):
            state[b, "QT"] = proj.tile([128, S], F32R, tag="QT", name="QT")
            state[b, "KT"] = proj.tile([128, S], F32R, tag="KT", name="KT")

        def kq_chunk(b, which, c, wtag, klo=0, khi=NKT):
            """proj chunk of 512 cols (k-range part); evac on DVE at khi==NKT."""
            qt_sb = state[b, "qt"]
            w_sb = wk_sb if which == "k" else wq_sb
            dst = state[b, "KT" if which == "k" else "QT"]
            sl = slice(c * 512, (c + 1) * 512)
            if klo == 0:
                state[b, "kqps", which] = ps_tile([128, 512], wtag)
            ps = state[b, "kqps", which]
            for k in range(klo, khi):
                nc.tensor.matmul(ps, w_sb[:, k, :], qt_sb[:, k, sl],
                                 start=(k == 0), stop=(k == NKT - 1))
            if khi == NKT:
                if which == "q":
                    nc.vector.tensor_scalar(
                        out=dst[:, sl], in0=ps, scalar1=bq_sb,
                        scalar2=eighth_sb, op0=Alu.add, op1=Alu.mult)
                else:
                    nc.vector.tensor_copy(dst[:, sl], ps)

        def v_st(b, st, wtag):
            """v-proj directly transposed: out [bs128, dpc128]."""
            qt_sb = state[b, "qt"]
            V = state[b, "V"]
            ps = ps_tile([128, 512], wtag)
            sl = slice(st * 128, (st + 1) * 128)
            for k in range(NKT):
                nc.tensor.matmul(ps[:, 0:128], qt_sb[:, k, sl], wv_sb[:, k, :],
                                 start=(k == 0), stop=(k == NKT - 1))
            for u in range(2):
                nc.vector.tensor_copy(V[:, st, u * 65:u * 65 + 64],
                                      ps[:, u * 64:(u + 1) * 64])

        def alloc_attn(b):
            state[b, "ctxT"] = ctp.tile([128, S], BF16, tag="ctxT", name="ctxT")

        def outproj_st(b, st, wtagA, wtagB):
            ctxT = state[b, "ctxT"]
            o_sb = outp.tile([128, D], BF16, tag="o", name="o_sb")
            for oc, wtag in ((0, wtagA), (1, wtagB)):
                ps = ps_tile([128, 512], wtag)
                nc.tensor.matmul(ps, ctxT[:, st * 128:(st + 1) * 128],
                                 wo_sb[:, oc * 512:(oc + 1) * 512],
                                 start=True, stop=True)
                nc.vector.tensor_copy(o_sb[:, oc * 512:(oc + 1) * 512], ps)
            nc.gpsimd.dma_start(
                out=out_d[b * S + st * 128: b * S + (st + 1) * 128, :], in_=o_sb)

        def attention_qc(b, qc, inserts, final=False):
            QT, KT, V = state[b, "QT"], state[b, "KT"], state[b, "V"]
            ctxT = state[b, "ctxT"]
            inserts = deque(inserts)
            e_tiles = {}
            pss = [None, None]

            for qt in range(8):
                state[b, qc, qt] = csbp.tile([128, 128], BF16, tag="csb",
                                             name="csb")
            ctx_ps = [ps_tile([128, 260], "cA"), ps_tile([128, 260], "cB")]

            def scores(u, sk):
                pss[u] = ps_tile([128, 1024], "sA" if u == 0 else "sB")
                for hh in range(2):
                    nc.tensor.matmul(
                        pss[u][:, hh * 512:(hh + 1) * 512],
                        KT[u * 64:(u + 1) * 64, sk * 128:(sk + 1) * 128],
                        QT[u * 64:(u + 1) * 64,
                           qc * 1024 + hh * 512: qc * 1024 + (hh + 1) * 512],
                        start=True, stop=True)

            def expop(u, sk):
                e = expp.tile([128, 1024], BF16, tag=f"e{u}",
                              bufs=(5 if u == 0 else 24), name=f"e{u}_t")
                nc.scalar.activation(e, pss[u], Act.Exp, bias=zero_sb, scale=1.0)
                e_tiles[u, sk] = e

            def ctx_mm(u, sk, qt, ps, col0):
                nc.tensor.matmul(
                    ps[:, col0:col0 + 65],
                    e_tiles[u, sk][:, qt * 128:(qt + 1) * 128],
                    V[:, sk, u * 65:u * 65 + 65],
                    start=(sk == 0), stop=(sk == NST - 1),
                    skip_group_check=True)

            def evac(u, qt, ps, col0):
                rcp = rcpp.tile([128, 1], F32, tag="rcp", name="rcp")
                nc.vector.reciprocal(rcp, ps[:, col0 + 64: col0 + 65])
                csb = state[b, qc, qt]
                nc.vector.tensor_scalar(
                    out=csb[:, u * 64:(u + 1) * 64],
                    in0=ps[:, col0: col0 + 64],
                    scalar1=rcp, scalar2=None, op0=Alu.mult)

            def u1_tail_qt(qt, wtag):
                """u1 ctx for one qt group through a w-tag; evac + transpose."""
                ps = ps_tile([128, 512], wtag)
                for sk in range(NST):
                    ctx_mm(1, sk, qt, ps, 0)
                evac(1, qt, ps, 0)
                csb = state[b, qc, qt]
                nc.sync.dma_start_transpose(
                    out=ctxT[:, qc * 1024 + qt * 128: qc * 1024 + (qt + 1) * 128],
                    in_=csb)

            scores(0, 0)
            scores(1, 0)
            for sk in range(NST):
                expop(0, sk)
                if sk + 1 < NST:
                    scores(0, sk + 1)
                for qt in range(8):
                    half, qtl = divmod(qt, 4)
                    ctx_mm(0, sk, qt, ctx_ps[half], qtl * 65)
                if inserts:
                    inserts.popleft()()
                expop(1, sk)
                if sk + 1 < NST:
                    scores(1, sk + 1)
                if inserts:
                    inserts.popleft()()
            # u0 normalize+evac (frees cA/cB for the next qc)
            for qt in range(8):
                half, qtl = divmod(qt, 4)
                evac(0, qt, ctx_ps[half], qtl * 65)

            tail = [functools.partial(u1_tail_qt, qt, next_w())
                    for qt in range(8)]
            if final:
                for qt in range(8):
                    tail[qt]()
                    outproj_st(b, 8 + qt, next_w(), next_w())
                tail = []
            return list(inserts), tail

        def thunk(f, *a):
            def g():
                f(*a)
            return g

        # =========================== schedule ===========================
        load_qt(0)
        load_qt(1)
        nc.sync.dma_start(out=wv_sb, in_=wv_d[:, :])
        alloc_proj(0)
        alloc_v(0)
        alloc_proj(1)
        alloc_v(1)
        alloc_attn(0)
        alloc_attn(1)

        # prologue: just enough for the first exp, then weave the rest
        kq_chunk(0, "k", 0, next_w())
        kq_chunk(0, "q", 0, next_w())
        kq_chunk(0, "q", 1, next_w())
        v_st(0, 0, next_w())
        v_st(0, 1, next_w())

        def kq_halves(b, which, c):
            w = next_w()
            return [thunk(kq_chunk, b, which, c, w, 0, 4),
                    thunk(kq_chunk, b, which, c, w, 4, NKT)]

        ins0 = []
        ins0 += kq_halves(0, "k", 1)
        ins0 += [thunk(v_st, 0, st, next_w()) for st in (2, 3, 4)]
        ins0 += kq_halves(0, "k", 2)
        ins0 += [thunk(v_st, 0, st, next_w()) for st in (5, 6)]
        ins0 += kq_halves(0, "k", 3)
        ins0 += [thunk(v_st, 0, st, next_w()) for st in (7, 8)]
        ins0 += kq_halves(0, "q", 2)
        ins0 += [thunk(v_st, 0, st, next_w()) for st in (9, 10)]
        ins0 += kq_halves(0, "q", 3)
        ins0 += [thunk(v_st, 0, st, next_w()) for st in (11, 12, 13, 14, 15)]
        ins0 += kq_halves(1, "k", 0)
        ins0 += kq_halves(1, "q", 0)
        left, tail0 = attention_qc(0, 0, ins0)

        ins1 = list(left) + list(tail0)
        ins1 += kq_halves(1, "k", 1)
        ins1 += kq_halves(1, "q", 1)
        ins1 += kq_halves(1, "k", 2)
        ins1 += kq_halves(1, "q", 2)
        ins1 += kq_halves(1, "k", 3)
        ins1 += kq_halves(1, "q", 3)
        left, tail1 = attention_qc(0, 1, ins1)

        # v(b1) first two inline (needed at steps 0/1 of b1-qc0)
        for thk in left:
            thk()
        v_st(1, 0, next_w())
        v_st(1, 1, next_w())
        ins2 = [thunk(v_st, 1, st, next_w()) for st in range(2, NST)]
        ins2 += list(tail1)
        ins2 += [thunk(outproj_st, 0, st, next_w(), next_w())
                 for st in range(8)]
        left, tail2 = attention_qc(1, 0, ins2)

        ins3 = list(left) + list(tail2)
        ins3 += [thunk(outproj_st, 0, st, next_w(), next_w())
                 for st in range(8, NST)]
        ins3 += [thunk(outproj_st, 1, st, next_w(), next_w())
                 for st in range(8)]
        left, _ = attention_qc(1, 1, ins3, final=True)
        for thk in left:
            thk()

    _split_sync_commands(nc)
    return nc


def _prepare(query, q_w, q_b, k_w, v_w, out_w):
    qt = np.ascontiguousarray(query.reshape(BS, D).T).astype(BF)  # [D, BS]

    def wprep(w, sl):
        # [D, DPC] -> [128, NKT*DPC]: partition = row within k-tile, free =
        # (k, dpc) contiguous, so the load is one fat DMA with 2KB rows.
        wt = np.ascontiguousarray(w[sl, :].T)          # [D, DPC]
        wt = wt.reshape(NKT, 128, DPC).transpose(1, 0, 2).reshape(128, NKT * DPC)
        return np.ascontiguousarray(wt).astype(BF)

    in_maps = []
    for c in range(N_CORES):
        sl = slice(c * DPC, (c + 1) * DPC)
        in_maps.append({
            "qt": qt,
            "wq": wprep(q_w, sl),
            "wk": wprep(k_w, sl),
            "wv": wprep(v_w, sl),
            "bq": np.ascontiguousarray(q_b[sl].reshape(DPC, 1)).astype(np.float32),
            "wo": np.ascontiguousarray(out_w[:, sl].T).astype(BF),
        })
    return in_maps


def kernel(query, mask, q_w, q_b, k_w, k_b, v_w, v_b, out_w, out_b):
    query = np.asarray(query, dtype=np.float32)
    q_w = np.asarray(q_w, dtype=np.float32); q_b = np.asarray(q_b, dtype=np.float32)
    k_w = np.asarray(k_w, dtype=np.float32)
    v_w = np.asarray(v_w, dtype=np.float32); v_b = np.asarray(v_b, dtype=np.float32)
    out_w = np.asarray(out_w, dtype=np.float32); out_b = np.asarray(out_b, dtype=np.float32)
    # k-bias cancels exactly in softmax (adds a per-query constant to all
    # scores of that query). v-bias adds a constant row to ctx (attention
    # rows sum to 1), contributing out_w @ v_b to every output row — folded
    # with out_b on the host.
    in_maps = _prepare(query, q_w, q_b, k_w, v_w, out_w)
    nc = _build()
    res = run_bass_kernel_spmd(nc, in_maps, core_ids=list(range(N_CORES)))
    out = np.zeros((BS, D), dtype=np.float32)
    for c in range(N_CORES):
        out += np.asarray(res.results[c]["out_part"], dtype=np.float32)
    out += (out_b + out_w @ v_b)[None, :]
    return out.reshape(B, S, D)


# revision 11
# speedup vs baseline: 1.4831x; 1.2714x over previous
"""v4: ACT-saturated schedule around the exp stream.

Cost-model-driven redesign vs v3 (see kernel_v3_baseline.py):
 - matmul cost = N(out free) x cycles_per_row(moving dtype); bf16 moving is
   1 cyc/row at any N (f32r needs N>=256). All HBM-sourced operands are
   pre-converted to bf16 on the host (halves load DMA too).
 - ctx matmul swapped: stationary = exp tile [keys,128q] (full 128x128),
   moving = V [keys, 64+ones] -> ctx cost halves; softmax denominator rides
   along as a ones column; normalization becomes a per-partition
   tensor_scalar at evac time.
 - V is projected directly transposed (stationary = qt tile, moving = wv):
   no PE transposes anywhere.
 - ctx^T for the out-projection via DMA-transpose (16x128 XBAR tiles).
 - k-bias dropped (exactly cancels in softmax), v-bias and out-bias folded
   on the host (attention rows sum to 1), q-bias folded into the QT evac.
 - ACT engine does nothing but the 128 exps (the roofline: ~133us); PE work
   of adjacent phases (proj, u1-ctx pass, outproj) is woven between score
   matmuls as cost-bounded inserts so the exp stream never starves. PE
   warmup matmuls defeat the p-state ramp.
 - PSUM (8 banks exactly): sA,sB [128,1024] (2+2), cA,cB [128,260] (1+1,
   u0 ctx: four 65-col qt groups each), w1,w2 [128,512] (1+1, rotating:
   warmup, k/q-proj chunks, v-proj tiles, u1-ctx qt groups, outproj halves).
"""

import functools
from collections import deque
from contextlib import ExitStack

import numpy as np
import ml_dtypes

import concourse.bass as bass
import concourse.tile as tile
from concourse import mybir
from concourse.bass_utils import run_bass_kernel_spmd

B, S, D, H, DH = 2, 2048, 1024, 16, 64
N_CORES = 8
DPC = D // N_CORES          # 128 channels/core = 2 heads
BS = B * S
NST = 16                    # key tiles of 128
NKT = 8                     # contraction tiles of 128

F32 = mybir.dt.float32
F32R = mybir.dt.float32r
BF16 = mybir.dt.bfloat16
Act = mybir.ActivationFunctionType
Alu = mybir.AluOpType
BF = ml_dtypes.bfloat16


def _split_sync_commands(nc, max_waits=1, max_updates=8):
    for fn in nc.m.functions:
        for bb in fn.blocks:
            new_insts = []
            changed = False
            for inst in bb.instructions:
                si = getattr(inst, "sync_info", None)
                if si is not None:
                    waits = list(si.on_wait or [])
                    if len(waits) > max_waits:
                        for w in waits[:-max_waits]:
                            new_insts.append(mybir.InstNoOp(
                                name=nc.get_next_instruction_name(),
                                ins=[], outs=[], engine=inst.engine,
                                sync_info=mybir.SyncInfo(on_wait=[w], on_update=[]),
                            ))
                        si.on_wait = waits[-max_waits:]
                        changed = True
                    updates = list(si.on_update or [])
                    if len(updates) > max_updates:
                        si.on_update = updates[:max_updates]
                        new_insts.append(inst)
                        new_insts.append(mybir.InstNoOp(
                            name=nc.get_next_instruction_name(),
                            ins=[], outs=[], engine=inst.engine,
                            sync_info=mybir.SyncInfo(
                                on_wait=[], on_update=updates[max_updates:]),
                        ))
                        changed = True
                        continue
                new_insts.append(inst)
            if changed:
                bb.instructions = new_insts


@functools.lru_cache(maxsize=1)
def _build():
    nc = bass.Bass()
    qt_d = nc.dram_tensor("qt", [D, BS], BF16, kind="ExternalInput")
    wq_d = nc.dram_tensor("wq", [128, NKT * DPC], BF16, kind="ExternalInput")
    wk_d = nc.dram_tensor("wk", [128, NKT * DPC], BF16, kind="ExternalInput")
    wv_d = nc.dram_tensor("wv", [128, NKT * DPC], BF16, kind="ExternalInput")
    bq_d = nc.dram_tensor("bq", [DPC, 1], F32, kind="ExternalInput")
    wo_d = nc.dram_tensor("wo", [DPC, D], BF16, kind="ExternalInput")
    out_d = nc.dram_tensor("out_part", [BS, D], BF16, kind="ExternalOutput")

    with tile.TileContext(nc) as tc, ExitStack() as ctx:
        consts = ctx.enter_context(tc.tile_pool(name="consts", bufs=1))
        qtp = ctx.enter_context(tc.tile_pool(name="qtp", bufs=1))
        proj = ctx.enter_context(tc.tile_pool(name="proj", bufs=2))
        vp = ctx.enter_context(tc.tile_pool(name="vp", bufs=2))
        expp = ctx.enter_context(tc.tile_pool(name="expp", bufs=1))
        csbp = ctx.enter_context(tc.tile_pool(name="csbp", bufs=8))
        ctp = ctx.enter_context(tc.tile_pool(name="ctp", bufs=2))
        rcpp = ctx.enter_context(tc.tile_pool(name="rcpp", bufs=4))
        outp = ctx.enter_context(tc.tile_pool(name="outp", bufs=4))
        psp = ctx.enter_context(tc.tile_pool(name="psp", bufs=1, space="PSUM"))

        def ps_tile(shape, tag):
            return psp.tile(shape, F32, tag=tag, name="ps_" + tag)

        _wrot = [0]

        def next_w():
            _wrot[0] ^= 1
            return "w1" if _wrot[0] else "w2"

        # ---------------- constants / warmup ----------------
        wconst = consts.tile([128, 640], BF16, tag="wconst")
        nc.vector.memset(wconst, 0.0)
        zero_sb = consts.tile([128, 1], F32, tag="zero")
        nc.vector.memset(zero_sb, 0.0)
        eighth_sb = consts.tile([128, 1], F32, tag="eighth")
        nc.vector.memset(eighth_sb, 0.125)

        for _ in range(11):
            ps = ps_tile([128, 512], next_w())
            nc.tensor.matmul(ps, wconst[:, 0:128], wconst[:, 128:640],
                             start=True, stop=True)

        # ---------------- weight / input loads ----------------
        # SP queue / DMA-device order = priority order: wk, qt-b0-c0, wq,
        # qt-c1, wv, qt-c2/3, qt-b1. bq/wo ride the ACT queue.
        wk_sb = consts.tile([128, NKT, DPC], BF16, tag="wk")
        wq_sb = consts.tile([128, NKT, DPC], BF16, tag="wq")
        wv_sb = consts.tile([128, NKT, DPC], BF16, tag="wv")
        bq_sb = consts.tile([128, 1], F32, tag="bq")
        wo_sb = consts.tile([128, D], BF16, tag="wo")

        state = {}

        def qt_chunk(b, c0, ncol=512):
            qt_sb = state[b, "qt"]
            qa = qt_d[:, :]
            nc.sync.dma_start(
                out=qt_sb[:, :, c0:c0 + ncol],
                in_=bass.AP(tensor=qa.tensor,
                            offset=qa.offset + b * S + c0,
                            ap=[[BS, 128], [128 * BS, NKT], [1, ncol]]))

        def alloc_qt(b):
            state[b, "qt"] = qtp.tile([128, NKT, S], BF16, tag=f"qt{b}",
                                      name=f"qt{b}")

        # V layout: [keys, st, 2*65]; cols u*65..u*65+63 = V_u, col u*65+64 = 1
        def alloc_v(b):
            V = vp.tile([128, NST, 130], BF16, tag="V", name="V")
            ones_ap = bass.AP(tensor=V.tensor, offset=V.offset + 64,
                              ap=[list(V.ap[0]), [130, NST], [65, 2], [1, 1]])
            nc.gpsimd.memset(ones_ap, 1.0)
            state[b, "V"] = V

        def alloc_proj(b):
            state[b, "QT"] = proj.tile([128, S], F32R, tag="QT", name="QT")
            state[b, "KT"] = proj.tile([128, S], F32R, tag="KT", name="KT")

        def kq_chunk(b, which, c, wtag, klo=0, khi=NKT):
            """proj chunk of 512 cols (k-range part); evac on DVE at khi==NKT."""
            qt_sb = state[b, "qt"]
            w_sb = wk_sb if which == "k" else wq_sb
            dst = state[b, "KT" if which == "k" else "QT"]
            sl = slice(c * 512, (c + 1) * 512)
            if klo == 0:
                state[b, "kqps", which] = ps_tile([128, 512], wtag)
            ps = state[b, "kqps", which]
            for k in range(klo, khi):
                nc.tensor.matmul(ps, w_sb[:, k, :], qt_sb[:, k, sl],
                                 start=(k == 0), stop=(k == NKT - 1))
            if khi == NKT:
                if which == "q":
                    nc.vector.tensor_scalar(
                        out=dst[:, sl], in0=ps, scalar1=bq_sb,
                        scalar2=eighth_sb, op0=Alu.add, op1=Alu.mult)
                else:
                    nc.vector.tensor_copy(dst[:, sl], ps)

        def v_st(b, st, wtag):
            """v-proj directly transposed: out [bs128, dpc128]."""
            qt_sb = state[b, "qt"]
            V = state[b, "V"]
            ps = ps_tile([128, 512], wtag)
            sl = slice(st * 128, (st + 1) * 128)
            for k in range(NKT):
                nc.tensor.matmul(ps[:, 0:128], qt_sb[:, k, sl], wv_sb[:, k, :],
                                 start=(k == 0), stop=(k == NKT - 1))
            for u in range(2):
                nc.vector.tensor_copy(V[:, st, u * 65:u * 65 + 64],
                                      ps[:, u * 64:(u + 1) * 64])

        def alloc_attn(b):
            state[b, "ctxT"] = ctp.tile([128, S], BF16, tag="ctxT", name="ctxT")

        def outproj_st(b, st, wtagA, wtagB, split_evac=False):
            ctxT = state[b, "ctxT"]
            o_sb = outp.tile([128, D], BF16, tag="o", name="o_sb")
            for oc, wtag in ((0, wtagA), (1, wtagB)):
                ps = ps_tile([128, 512], wtag)
                nc.tensor.matmul(ps, ctxT[:, st * 128:(st + 1) * 128],
                                 wo_sb[:, oc * 512:(oc + 1) * 512],
                                 start=True, stop=True)
                if split_evac and oc == 1:
                    # ACT is idle after the last exp; GPSIMD can't read PSUM
                    nc.scalar.activation(o_sb[:, oc * 512:(oc + 1) * 512],
                                         ps, Act.Copy, bias=0.0, scale=1.0)
                else:
                    nc.vector.tensor_copy(o_sb[:, oc * 512:(oc + 1) * 512], ps)
            nc.gpsimd.dma_start(
                out=out_d[b * S + st * 128: b * S + (st + 1) * 128, :], in_=o_sb)

        def attention_qc(b, qc, inserts, final=False):
            QT, KT, V = state[b, "QT"], state[b, "KT"], state[b, "V"]
            ctxT = state[b, "ctxT"]
            inserts = deque(inserts)
            e_tiles = {}
            pss = [None, None]

            for qt in range(8):
                state[b, qc, qt] = csbp.tile([128, 128], BF16, tag="csb",
                                             name="csb")
            ctx_ps = [ps_tile([128, 260], "cA"), ps_tile([128, 260], "cB")]
            nc.vector.memset(ctx_ps[0], 0.0)
            nc.vector.memset(ctx_ps[1], 0.0)

            def scores(u, sk):
                pss[u] = ps_tile([128, 1024], "sA" if u == 0 else "sB")
                for hh in range(2):
                    nc.tensor.matmul(
                        pss[u][:, hh * 512:(hh + 1) * 512],
                        KT[u * 64:(u + 1) * 64, sk * 128:(sk + 1) * 128],
                        QT[u * 64:(u + 1) * 64,
                           qc * 1024 + hh * 512: qc * 1024 + (hh + 1) * 512],
                        start=True, stop=True)

            def expop(u, sk):
                e = expp.tile([128, 1024], BF16, tag=f"e{u}",
                              bufs=(5 if u == 0 else 24), name=f"e{u}_t")
                nc.scalar.activation(e, pss[u], Act.Exp, bias=zero_sb, scale=1.0)
                e_tiles[u, sk] = e

            def ctx_mm(u, sk, qt, ps, col0, multigroup=True):
                # multigroup tiles (cA/cB) hold 4 qt groups per bank; a
                # start=True would zero the whole bank on HW, so those are
                # DVE-memset instead and always accumulate.
                nc.tensor.matmul(
                    ps[:, col0:col0 + 65],
                    e_tiles[u, sk][:, qt * 128:(qt + 1) * 128],
                    V[:, sk, u * 65:u * 65 + 65],
                    start=(sk == 0 and not multigroup),
                    stop=(sk == NST - 1),
                    skip_group_check=True)

            def evac(u, qt, ps, col0):
                rcp = rcpp.tile([128, 1], F32, tag="rcp", name="rcp")
                nc.vector.reciprocal(rcp, ps[:, col0 + 64: col0 + 65])
                csb = state[b, qc, qt]
                nc.vector.tensor_scalar(
                    out=csb[:, u * 64:(u + 1) * 64],
                    in0=ps[:, col0: col0 + 64],
                    scalar1=rcp, scalar2=None, op0=Alu.mult)

            def u1_tail_qt(qt, wtag):
                """u1 ctx for one qt group through a w-tag; evac + transpose."""
                ps = ps_tile([128, 512], wtag)
                for sk in range(NST):
                    ctx_mm(1, sk, qt, ps, 0, multigroup=False)
                evac(1, qt, ps, 0)
                csb = state[b, qc, qt]
                nc.sync.dma_start_transpose(
                    out=ctxT[:, qc * 1024 + qt * 128: qc * 1024 + (qt + 1) * 128],
                    in_=csb)

            def run_inserts(budget):
                while inserts and budget > 0:
                    cost, fn = inserts[0]
                    inserts.popleft()
                    fn()
                    budget -= cost
                return budget

            scores(0, 0)
            scores(1, 0)
            for sk in range(NST):
                expop(0, sk)
                if sk + 1 < NST:
                    scores(0, sk + 1)
                for qt in range(8):
                    half, qtl = divmod(qt, 4)
                    ctx_mm(0, sk, qt, ctx_ps[half], qtl * 65)
                rem = run_inserts(500)
                expop(1, sk)
                if sk + 1 < NST:
                    scores(1, sk + 1)
                run_inserts(rem + 400)
            # u0 normalize+evac (frees cA/cB for the next qc)
            for qt in range(8):
                half, qtl = divmod(qt, 4)
                evac(0, qt, ctx_ps[half], qtl * 65)

            tail = [(450, functools.partial(u1_tail_qt, qt, next_w()))
                    for qt in range(8)]
            if final:
                # software-pipelined finale: all ctx chains + transposes
                # first, then the outprojs (on the now-free cA/cB banks).
                for _, fn in tail:
                    fn()
                for qt in range(8):
                    outproj_st(b, 8 + qt, "cA", "cB", split_evac=True)
                tail = []
            return list(inserts), tail

        def thunk(f, *a):
            def g():
                f(*a)
            return g

        # =========================== schedule ===========================
        alloc_qt(0)
        alloc_qt(1)
        nc.sync.dma_start(out=wk_sb, in_=wk_d[:, :])
        qt_chunk(0, 0)
        nc.sync.dma_start(out=wq_sb, in_=wq_d[:, :])
        qt_chunk(0, 512)
        nc.sync.dma_start(out=wv_sb, in_=wv_d[:, :])
        qt_chunk(0, 1024)
        qt_chunk(0, 1536)
        for c0 in range(0, S, 512):
            qt_chunk(1, c0)
        nc.scalar.dma_start(out=bq_sb, in_=bq_d[:, :])
        nc.scalar.dma_start(out=wo_sb, in_=wo_d[:, :])
        alloc_proj(0)
        alloc_v(0)
        alloc_proj(1)
        alloc_v(1)
        alloc_attn(0)
        alloc_attn(1)

        # prologue: just enough for the first exp, then weave the rest
        kq_chunk(0, "k", 0, next_w())
        kq_chunk(0, "q", 0, next_w())
        kq_chunk(0, "q", 1, next_w())
        v_st(0, 0, next_w())
        v_st(0, 1, next_w())

        def kq_halves(b, which, c):
            w = next_w()
            return [(430, thunk(kq_chunk, b, which, c, w, 0, 4)),
                    (430, thunk(kq_chunk, b, which, c, w, 4, NKT))]

        def v_thunks(b, sts):
            return [(430, thunk(v_st, b, st, next_w())) for st in sts]

        def op_thunks(b, sts):
            return [(640, thunk(outproj_st, b, st, next_w(), next_w()))
                    for st in sts]

        ins0 = []
        ins0 += kq_halves(0, "k", 1)
        ins0 += v_thunks(0, (2, 3, 4))
        ins0 += kq_halves(0, "k", 2)
        ins0 += v_thunks(0, (5, 6))
        ins0 += kq_halves(0, "k", 3)
        ins0 += v_thunks(0, (7, 8))
        ins0 += kq_halves(0, "q", 2)
        ins0 += v_thunks(0, (9, 10))
        ins0 += kq_halves(0, "q", 3)
        ins0 += v_thunks(0, (11, 12, 13, 14, 15))
        ins0 += kq_halves(1, "k", 0)
        ins0 += kq_halves(1, "q", 0)
        left, tail0 = attention_qc(0, 0, ins0)

        ins1 = list(left) + list(tail0)
        ins1 += kq_halves(1, "k", 1)
        ins1 += kq_halves(1, "q", 1)
        ins1 += kq_halves(1, "k", 2)
        ins1 += kq_halves(1, "q", 2)
        ins1 += kq_halves(1, "k", 3)
        ins1 += kq_halves(1, "q", 3)
        left, tail1 = attention_qc(0, 1, ins1)

        # v(b1) first two inline (needed at steps 0/1 of b1-qc0)
        for _, fn in left:
            fn()
        v_st(1, 0, next_w())
        v_st(1, 1, next_w())
        ins2 = v_thunks(1, range(2, NST))
        ins2 += list(tail1)
        ins2 += op_thunks(0, range(8))
        left, tail2 = attention_qc(1, 0, ins2)

        ins3 = list(left) + list(tail2)
        ins3 += op_thunks(0, range(8, NST))
        ins3 += op_thunks(1, range(8))
        left, _ = attention_qc(1, 1, ins3, final=True)
        for _, fn in left:
            fn()

    _split_sync_commands(nc)
    return nc


def _prepare(query, q_w, q_b, k_w, v_w, out_w):
    qt = np.ascontiguousarray(query.reshape(BS, D).T).astype(BF)  # [D, BS]

    def wprep(w, sl):
        # [D, DPC] -> [128, NKT*DPC]: partition = row within k-tile, free =
        # (k, dpc) contiguous, so the load is one fat DMA with 2KB rows.
        wt = np.ascontiguousarray(w[sl, :].T)          # [D, DPC]
        wt = wt.reshape(NKT, 128, DPC).transpose(1, 0, 2).reshape(128, NKT * DPC)
        return np.ascontiguousarray(wt).astype(BF)

    in_maps = []
    for c in range(N_CORES):
        sl = slice(c * DPC, (c + 1) * DPC)
        in_maps.append({
            "qt": qt,
            "wq": wprep(q_w, sl),
            "wk": wprep(k_w, sl),
            "wv": wprep(v_w, sl),
            "bq": np.ascontiguousarray(q_b[sl].reshape(DPC, 1)).astype(np.float32),
            "wo": np.ascontiguousarray(out_w[:, sl].T).astype(BF),
        })
    return in_maps


def kernel(query, mask, q_w, q_b, k_w, k_b, v_w, v_b, out_w, out_b):
    query = np.asarray(query, dtype=np.float32)
    q_w = np.asarray(q_w, dtype=np.float32); q_b = np.asarray(q_b, dtype=np.float32)
    k_w = np.asarray(k_w, dtype=np.float32)
    v_w = np.asarray(v_w, dtype=np.float32); v_b = np.asarray(v_b, dtype=np.float32)
    out_w = np.asarray(out_w, dtype=np.float32); out_b = np.asarray(out_b, dtype=np.float32)
    # k-bias cancels exactly in softmax (adds a per-query constant to all
    # scores of that query). v-bias adds a constant row to ctx (attention
    # rows sum to 1), contributing out_w @ v_b to every output row — folded
    # with out_b on the host.
    in_maps = _prepare(query, q_w, q_b, k_w, v_w, out_w)
    nc = _build()
    res = run_bass_kernel_spmd(nc, in_maps, core_ids=list(range(N_CORES)))
    out = np.zeros((BS, D), dtype=np.float32)
    for c in range(N_CORES):
        out += np.asarray(res.results[c]["out_part"], dtype=np.float32)
    out += (out_b + out_w @ v_b)[None, :]
    return out.reshape(B, S, D)


# revision 14
# speedup vs baseline: 1.5538x; 1.0477x over previous
"""v4: ACT-saturated schedule around the exp stream.

Cost-model-driven redesign vs v3 (see kernel_v3_baseline.py):
 - matmul cost = N(out free) x cycles_per_row(moving dtype); bf16 moving is
   1 cyc/row at any N (f32r needs N>=256). All HBM-sourced operands are
   pre-converted to bf16 on the host (halves load DMA too).
 - ctx matmul swapped: stationary = exp tile [keys,128q] (full 128x128),
   moving = V [keys, 64+ones] -> ctx cost halves; softmax denominator rides
   along as a ones column; normalization becomes a per-partition
   tensor_scalar at evac time.
 - V is projected directly transposed (stationary = qt tile, moving = wv):
   no PE transposes anywhere.
 - ctx^T for the out-projection via DMA-transpose (16x128 XBAR tiles).
 - k-bias dropped (exactly cancels in softmax), v-bias and out-bias folded
   on the host (attention rows sum to 1), q-bias folded into the QT evac.
 - ACT engine does nothing but the 128 exps (the roofline: ~133us); PE work
   of adjacent phases (proj, u1-ctx pass, outproj) is woven between score
   matmuls as cost-bounded inserts so the exp stream never starves. PE
   warmup matmuls defeat the p-state ramp.
 - PSUM (8 banks exactly): sA,sB [128,1024] (2+2), cA,cB [128,260] (1+1,
   u0 ctx: four 65-col qt groups each), w1,w2 [128,512] (1+1, rotating:
   warmup, k/q-proj chunks, v-proj tiles, u1-ctx qt groups, outproj halves).
"""

import functools
from collections import deque
from contextlib import ExitStack

import numpy as np
import ml_dtypes

import concourse.bass as bass
import concourse.tile as tile
from concourse import mybir
from concourse.bass_utils import run_bass_kernel_spmd

B, S, D, H, DH = 2, 2048, 1024, 16, 64
N_CORES = 8
DPC = D // N_CORES          # 128 channels/core = 2 heads
BS = B * S
NST = 16                    # key tiles of 128
NKT = 8                     # contraction tiles of 128

F32 = mybir.dt.float32
F32R = mybir.dt.float32r
BF16 = mybir.dt.bfloat16
Act = mybir.ActivationFunctionType
Alu = mybir.AluOpType
BF = ml_dtypes.bfloat16


def _split_sync_commands(nc, max_waits=1, max_updates=8):
    for fn in nc.m.functions:
        for bb in fn.blocks:
            new_insts = []
            changed = False
            for inst in bb.instructions:
                si = getattr(inst, "sync_info", None)
                if si is not None:
                    waits = list(si.on_wait or [])
                    if len(waits) > max_waits:
                        for w in waits[:-max_waits]:
                            new_insts.append(mybir.InstNoOp(
                                name=nc.get_next_instruction_name(),
                                ins=[], outs=[], engine=inst.engine,
                                sync_info=mybir.SyncInfo(on_wait=[w], on_update=[]),
                            ))
                        si.on_wait = waits[-max_waits:]
                        changed = True
                    updates = list(si.on_update or [])
                    if len(updates) > max_updates:
                        si.on_update = updates[:max_updates]
                        new_insts.append(inst)
                        new_insts.append(mybir.InstNoOp(
                            name=nc.get_next_instruction_name(),
                            ins=[], outs=[], engine=inst.engine,
                            sync_info=mybir.SyncInfo(
                                on_wait=[], on_update=updates[max_updates:]),
                        ))
                        changed = True
                        continue
                new_insts.append(inst)
            if changed:
                bb.instructions = new_insts


@functools.lru_cache(maxsize=1)
def _build():
    nc = bass.Bass()
    qt_d = nc.dram_tensor("qt", [D, BS], BF16, kind="ExternalInput")
    wq_d = nc.dram_tensor("wq", [128, NKT * DPC], BF16, kind="ExternalInput")
    wk_d = nc.dram_tensor("wk", [128, NKT * DPC], BF16, kind="ExternalInput")
    wv_d = nc.dram_tensor("wv", [128, NKT * DPC], BF16, kind="ExternalInput")
    bq_d = nc.dram_tensor("bq", [DPC, 1], F32, kind="ExternalInput")
    wo_d = nc.dram_tensor("wo", [DPC, D], BF16, kind="ExternalInput")
    out_d = nc.dram_tensor("out_part", [BS, D], BF16, kind="ExternalOutput")

    with tile.TileContext(nc) as tc, ExitStack() as ctx:
        consts = ctx.enter_context(tc.tile_pool(name="consts", bufs=1))
        qtp = ctx.enter_context(tc.tile_pool(name="qtp", bufs=1))
        proj = ctx.enter_context(tc.tile_pool(name="proj", bufs=2))
        vp = ctx.enter_context(tc.tile_pool(name="vp", bufs=2))
        expp = ctx.enter_context(tc.tile_pool(name="expp", bufs=1))
        csbp = ctx.enter_context(tc.tile_pool(name="csbp", bufs=8))
        ctp = ctx.enter_context(tc.tile_pool(name="ctp", bufs=2))
        rcpp = ctx.enter_context(tc.tile_pool(name="rcpp", bufs=4))
        outp = ctx.enter_context(tc.tile_pool(name="outp", bufs=4))
        psp = ctx.enter_context(tc.tile_pool(name="psp", bufs=1, space="PSUM"))

        def ps_tile(shape, tag):
            return psp.tile(shape, F32, tag=tag, name="ps_" + tag)

        _wrot = [0]

        def next_w():
            _wrot[0] ^= 1
            return "w1" if _wrot[0] else "w2"

        # ---------------- constants / warmup ----------------
        wconst = consts.tile([128, 640], BF16, tag="wconst")
        nc.vector.memset(wconst, 0.0)
        zero_sb = consts.tile([128, 1], F32, tag="zero")
        nc.vector.memset(zero_sb, 0.0)
        eighth_sb = consts.tile([128, 1], F32, tag="eighth")
        nc.vector.memset(eighth_sb, 0.125)

        for _ in range(11):
            ps = ps_tile([128, 512], next_w())
            nc.tensor.matmul(ps, wconst[:, 0:128], wconst[:, 128:640],
                             start=True, stop=True)

        # ---------------- weight / input loads ----------------
        # SP queue / DMA-device order = priority order: wk, qt-b0-c0, wq,
        # qt-c1, wv, qt-c2/3, qt-b1. bq/wo ride the ACT queue.
        wk_sb = consts.tile([128, NKT, DPC], BF16, tag="wk")
        wq_sb = consts.tile([128, NKT, DPC], BF16, tag="wq")
        wv_sb = consts.tile([128, NKT, DPC], BF16, tag="wv")
        bq_sb = consts.tile([128, 1], F32, tag="bq")
        wo_sb = consts.tile([128, D], BF16, tag="wo")
        ident_d = nc.inline_tensor(
            np.eye(128, dtype=np.float32).astype(ml_dtypes.bfloat16), "identb")
        ident_sb = consts.tile([128, 128], BF16, tag="ident")

        state = {}

        def qt_chunk(b, c0, ncol=512):
            qt_sb = state[b, "qt"]
            qa = qt_d[:, :]
            nc.sync.dma_start(
                out=qt_sb[:, :, c0:c0 + ncol],
                in_=bass.AP(tensor=qa.tensor,
                            offset=qa.offset + b * S + c0,
                            ap=[[BS, 128], [128 * BS, NKT], [1, ncol]]))

        def alloc_qt(b):
            state[b, "qt"] = qtp.tile([128, NKT, S], BF16, tag=f"qt{b}",
                                      name=f"qt{b}")

        # V layout: [keys, st, 2*65]; cols u*65..u*65+63 = V_u, col u*65+64 = 1
        def alloc_v(b):
            V = vp.tile([128, NST, 130], BF16, tag="V", name="V")
            ones_ap = bass.AP(tensor=V.tensor, offset=V.offset + 64,
                              ap=[list(V.ap[0]), [130, NST], [65, 2], [1, 1]])
            nc.gpsimd.memset(ones_ap, 1.0)
            state[b, "V"] = V

        def alloc_proj(b):
            state[b, "QT"] = proj.tile([128, S], F32R, tag="QT", name="QT")
            state[b, "KT"] = proj.tile([128, S], F32R, tag="KT", name="KT")

        def kq_chunk(b, which, c, wtag, klo=0, khi=NKT, c0=None, ncol=512):
            """proj chunk (k-range part); evac on DVE at khi==NKT."""
            qt_sb = state[b, "qt"]
            w_sb = wk_sb if which == "k" else wq_sb
            dst = state[b, "KT" if which == "k" else "QT"]
            if c0 is None:
                c0 = c * 512
            sl = slice(c0, c0 + ncol)
            if klo == 0:
                state[b, "kqps", which] = ps_tile([128, 512], wtag)
            ps = state[b, "kqps", which]
            for k in range(klo, khi):
                nc.tensor.matmul(ps[:, 0:ncol], w_sb[:, k, :], qt_sb[:, k, sl],
                                 start=(k == 0), stop=(k == NKT - 1))
            if khi == NKT:
                if which == "q":
                    nc.vector.tensor_scalar(
                        out=dst[:, sl], in0=ps[:, 0:ncol], scalar1=bq_sb,
                        scalar2=eighth_sb, op0=Alu.add, op1=Alu.mult)
                else:
                    nc.vector.tensor_copy(dst[:, sl], ps[:, 0:ncol])

        def v_st(b, st, wtag):
            """v-proj directly transposed: out [bs128, dpc128]."""
            qt_sb = state[b, "qt"]
            V = state[b, "V"]
            ps = ps_tile([128, 512], wtag)
            sl = slice(st * 128, (st + 1) * 128)
            for k in range(NKT):
                nc.tensor.matmul(ps[:, 0:128], qt_sb[:, k, sl], wv_sb[:, k, :],
                                 start=(k == 0), stop=(k == NKT - 1))
            for u in range(2):
                nc.vector.tensor_copy(V[:, st, u * 65:u * 65 + 64],
                                      ps[:, u * 64:(u + 1) * 64])

        def alloc_attn(b):
            state[b, "ctxT"] = ctp.tile([128, S], BF16, tag="ctxT", name="ctxT")

        def outproj_st(b, st, wtagA, wtagB, split_evac=False):
            # adjacent st pairs share one [128, 2, 1024] tile and one store
            # DMA over 256 contiguous DRAM rows (halves Pool SWDGE issue cost)
            ctxT = state[b, "ctxT"]
            g = st % 2
            if g == 0:
                state[b, "opair"] = outp.tile([128, 2, D], BF16, tag="o",
                                              name="o_sb")
            o_sb = state[b, "opair"]
            for oc, wtag in ((0, wtagA), (1, wtagB)):
                ps = ps_tile([128, 512], wtag)
                nc.tensor.matmul(ps, ctxT[:, st * 128:(st + 1) * 128],
                                 wo_sb[:, oc * 512:(oc + 1) * 512],
                                 start=True, stop=True)
                if split_evac and oc == 1:
                    # ACT is idle after the last exp; GPSIMD can't read PSUM
                    nc.scalar.activation(o_sb[:, g, oc * 512:(oc + 1) * 512],
                                         ps, Act.Copy, bias=0.0, scale=1.0)
                else:
                    nc.vector.tensor_copy(o_sb[:, g, oc * 512:(oc + 1) * 512],
                                          ps)
            if g == 1:
                r0 = b * S + (st - 1) * 128
                oa = out_d[r0:r0 + 256, :]
                nc.gpsimd.dma_start(
                    out=bass.AP(tensor=oa.tensor, offset=oa.offset,
                                ap=[[D, 128], [128 * D, 2], [1, D]]),
                    in_=o_sb)

        def attention_qc(b, qc, inserts, final=False, halves=False):
            QT, KT, V = state[b, "QT"], state[b, "KT"], state[b, "V"]
            ctxT = state[b, "ctxT"]
            inserts = deque(inserts)
            e_tiles = {}
            pss = [None, None]

            for qt in range(8):
                state[b, qc, qt] = csbp.tile([128, 128], BF16, tag="csb",
                                             name="csb")
            ctx_ps = [ps_tile([128, 260], "cA"), ps_tile([128, 260], "cB")]
            nc.vector.memset(ctx_ps[0], 0.0)
            nc.vector.memset(ctx_ps[1], 0.0)

            def scores(u, sk):
                pss[u] = ps_tile([128, 1024], "sA" if u == 0 else "sB")
                for hh in range(2):
                    nc.tensor.matmul(
                        pss[u][:, hh * 512:(hh + 1) * 512],
                        KT[u * 64:(u + 1) * 64, sk * 128:(sk + 1) * 128],
                        QT[u * 64:(u + 1) * 64,
                           qc * 1024 + hh * 512: qc * 1024 + (hh + 1) * 512],
                        start=True, stop=True)

            def expop(u, sk, halves=False):
                e = expp.tile([128, 1024], BF16, tag=f"e{u}",
                              bufs=(5 if u == 0 else 24), name=f"e{u}_t")
                if halves:
                    nc.scalar.activation(e[:, 0:512], pss[u][:, 0:512],
                                         Act.Exp, bias=zero_sb, scale=1.0)
                    nc.scalar.activation(e[:, 512:1024], pss[u][:, 512:1024],
                                         Act.Exp, bias=zero_sb, scale=1.0)
                else:
                    nc.scalar.activation(e, pss[u], Act.Exp, bias=zero_sb,
                                         scale=1.0)
                e_tiles[u, sk] = e

            def ctx_mm(u, sk, qt, ps, col0, multigroup=True):
                # multigroup tiles (cA/cB) hold 4 qt groups per bank; a
                # start=True would zero the whole bank on HW, so those are
                # DVE-memset instead and always accumulate.
                nc.tensor.matmul(
                    ps[:, col0:col0 + 65],
                    e_tiles[u, sk][:, qt * 128:(qt + 1) * 128],
                    V[:, sk, u * 65:u * 65 + 65],
                    start=(sk == 0 and not multigroup),
                    stop=(sk == NST - 1),
                    skip_group_check=True)

            def evac(u, qt, ps, col0, on_act=False):
                rcp = rcpp.tile([128, 1], F32, tag="rcp", name="rcp")
                nc.vector.reciprocal(rcp, ps[:, col0 + 64: col0 + 65])
                csb = state[b, qc, qt]
                if on_act:
                    nc.scalar.activation(
                        csb[:, u * 64:(u + 1) * 64], ps[:, col0: col0 + 64],
                        Act.Copy, bias=0.0, scale=rcp)
                else:
                    nc.vector.tensor_scalar(
                        out=csb[:, u * 64:(u + 1) * 64],
                        in0=ps[:, col0: col0 + 64],
                        scalar1=rcp, scalar2=None, op0=Alu.mult)

            def u1_tail_qt(qt, wtag):
                """u1 ctx for one qt group through a w-tag; evac + transpose."""
                ps = ps_tile([128, 512], wtag)
                for sk in range(NST):
                    ctx_mm(1, sk, qt, ps, 0, multigroup=False)
                evac(1, qt, ps, 0)
                csb = state[b, qc, qt]
                sl = slice(qc * 1024 + qt * 128, qc * 1024 + (qt + 1) * 128)
                if final:
                    # post-stream: PE transpose into the free sA bank, ACT
                    # (also free) evacuates to ctxT. No HWDGE / DMA sems.
                    if ("trps",) not in state:
                        state["trps",] = psp.tile([128, 8, 128], BF16,
                                                  tag="sA", name="trps")
                    trps = state["trps",]
                    nc.tensor.transpose(trps[:, qt, :], csb, ident_sb)
                    nc.scalar.activation(ctxT[:, sl], trps[:, qt, :],
                                         Act.Copy, bias=0.0, scale=1.0)
                else:
                    nc.sync.dma_start_transpose(out=ctxT[:, sl], in_=csb)

            def run_inserts(budget, force_first=False):
                while inserts and (inserts[0][0] <= budget or force_first):
                    force_first = False
                    cost, fn = inserts.popleft()
                    fn()
                    budget -= cost
                return budget

            scores(0, 0)
            scores(1, 0)
            for sk in range(NST):
                expop(0, sk, halves=halves and sk == 0)
                if sk + 1 < NST:
                    scores(0, sk + 1)
                for qt in range(8):
                    half, qtl = divmod(qt, 4)
                    ctx_mm(0, sk, qt, ctx_ps[half], qtl * 65)
                rem = run_inserts(520, force_first=True)
                expop(1, sk, halves=halves and sk == 0)
                if sk + 1 < NST:
                    scores(1, sk + 1)
                run_inserts(rem + 430)
            # u0 normalize+evac (frees cA/cB for the next qc)
            for qt in range(8):
                half, qtl = divmod(qt, 4)
                evac(0, qt, ctx_ps[half], qtl * 65,
                     on_act=(final and qt % 2 == 1))

            tail = [(450, functools.partial(u1_tail_qt, qt, next_w()))
                    for qt in range(8)]
            if final:
                # software-pipelined finale: all ctx chains + transposes
                # first, then the outprojs (on the now-free cA/cB banks).
                for _, fn in tail:
                    fn()
                optags = [("cA", "cB"), ("w1", "w2")]
                for qt in range(8):
                    ta, tb = optags[qt % 2]
                    outproj_st(b, 8 + qt, ta, tb, split_evac=True)
                tail = []
            return list(inserts), tail

        def thunk(f, *a):
            def g():
                f(*a)
            return g

        # =========================== schedule ===========================
        alloc_qt(0)
        alloc_qt(1)
        nc.scalar.dma_start(out=bq_sb, in_=bq_d[:, :])
        nc.sync.dma_start(out=wk_sb, in_=wk_d[:, :])
        qt_chunk(0, 0)
        nc.sync.dma_start(out=wq_sb, in_=wq_d[:, :])
        qt_chunk(0, 512, ncol=256)
        qt_chunk(0, 768, ncol=256)
        nc.sync.dma_start(out=wv_sb, in_=wv_d[:, :])
        qt_chunk(0, 1024)
        qt_chunk(0, 1536)
        nc.scalar.dma_start(out=wo_sb, in_=wo_d[:, :])
        nc.scalar.dma_start(out=ident_sb, in_=ident_d[:, :])
        for c0 in range(0, S, 512):
            qt_chunk(1, c0)
        alloc_proj(0)
        alloc_v(0)
        alloc_proj(1)
        alloc_v(1)
        alloc_attn(0)
        alloc_attn(1)

        # prologue: just enough for the first exp, then weave the rest
        kq_chunk(0, "k", 0, next_w())
        kq_chunk(0, "q", 0, next_w())
        kq_chunk(0, "q", None, next_w(), c0=512, ncol=256)
        kq_chunk(0, "q", None, next_w(), c0=768, ncol=256)
        v_st(0, 0, next_w())
        v_st(0, 1, next_w())

        def kq_halves(b, which, c):
            w = next_w()
            return [(430, thunk(kq_chunk, b, which, c, w, 0, 4)),
                    (430, thunk(kq_chunk, b, which, c, w, 4, NKT))]

        def v_thunks(b, sts):
            return [(430, thunk(v_st, b, st, next_w())) for st in sts]

        def op_thunks(b, sts):
            return [(640, thunk(outproj_st, b, st, next_w(), next_w()))
                    for st in sts]

        ins0 = []
        ins0 += kq_halves(0, "k", 1)
        ins0 += v_thunks(0, (2, 3, 4))
        ins0 += kq_halves(0, "k", 2)
        ins0 += v_thunks(0, (5, 6))
        ins0 += kq_halves(0, "k", 3)
        ins0 += v_thunks(0, (7, 8))
        ins0 += kq_halves(0, "q", 2)
        ins0 += v_thunks(0, (9, 10))
        ins0 += kq_halves(0, "q", 3)
        ins0 += v_thunks(0, (11, 12, 13, 14, 15))
        ins0 += kq_halves(1, "k", 0)
        ins0 += kq_halves(1, "q", 0)
        left, tail0 = attention_qc(0, 0, ins0, halves=True)

        ins1 = list(left) + list(tail0)
        ins1 += kq_halves(1, "k", 1)
        ins1 += kq_halves(1, "q", 1)
        ins1 += kq_halves(1, "k", 2)
        ins1 += kq_halves(1, "q", 2)
        ins1 += kq_halves(1, "k", 3)
        ins1 += kq_halves(1, "q", 3)
        ins1 += v_thunks(1, range(2, 8))
        ins1 += op_thunks(0, range(0, 2))
        left, tail1 = attention_qc(0, 1, ins1)

        # v(b1) first two inline (needed at steps 0/1 of b1-qc0)
        for _, fn in left:
            fn()
        v_st(1, 0, next_w())
        v_st(1, 1, next_w())
        ins2 = v_thunks(1, range(8, NST))
        ins2 += list(tail1)
        ins2 += op_thunks(0, range(2, 10))
        left, tail2 = attention_qc(1, 0, ins2)

        ins3 = list(left) + list(tail2)
        ins3 += op_thunks(0, range(10, NST))
        ins3 += op_thunks(1, range(8))
        left, _ = attention_qc(1, 1, ins3, final=True)
        for _, fn in left:
            fn()

    _split_sync_commands(nc)
    return nc


def _prepare(query, q_w, q_b, k_w, v_w, out_w):
    qt = np.ascontiguousarray(query.reshape(BS, D).T).astype(BF)  # [D, BS]

    def wprep(w, sl):
        # [D, DPC] -> [128, NKT*DPC]: partition = row within k-tile, free =
        # (k, dpc) contiguous, so the load is one fat DMA with 2KB rows.
        wt = np.ascontiguousarray(w[sl, :].T)          # [D, DPC]
        wt = wt.reshape(NKT, 128, DPC).transpose(1, 0, 2).reshape(128, NKT * DPC)
        return np.ascontiguousarray(wt).astype(BF)

    in_maps = []
    for c in range(N_CORES):
        sl = slice(c * DPC, (c + 1) * DPC)
        in_maps.append({
            "qt": qt,
            "wq": wprep(q_w, sl),
            "wk": wprep(k_w, sl),
            "wv": wprep(v_w, sl),
            "bq": np.ascontiguousarray(q_b[sl].reshape(DPC, 1)).astype(np.float32),
            "wo": np.ascontiguousarray(out_w[:, sl].T).astype(BF),
        })
    return in_maps


def kernel(query, mask, q_w, q_b, k_w, k_b, v_w, v_b, out_w, out_b):
    query = np.asarray(query, dtype=np.float32)
    q_w = np.asarray(q_w, dtype=np.float32); q_b = np.asarray(q_b, dtype=np.float32)
    k_w = np.asarray(k_w, dtype=np.float32)
    v_w = np.asarray(v_w, dtype=np.float32); v_b = np.asarray(v_b, dtype=np.float32)
    out_w = np.asarray(out_w, dtype=np.float32); out_b = np.asarray(out_b, dtype=np.float32)
    # k-bias cancels exactly in softmax (adds a per-query constant to all
    # scores of that query). v-bias adds a constant row to ctx (attention
    # rows sum to 1), contributing out_w @ v_b to every output row — folded
    # with out_b on the host.
    in_maps = _prepare(query, q_w, q_b, k_w, v_w, out_w)
    nc = _build()
    res = run_bass_kernel_spmd(nc, in_maps, core_ids=list(range(N_CORES)))
    out = np.zeros((BS, D), dtype=np.float32)
    for c in range(N_CORES):
        out += np.asarray(res.results[c]["out_part"], dtype=np.float32)
    out += (out_b + out_w @ v_b)[None, :]
    return out.reshape(B, S, D)
